# revision 39
# baseline (speedup 1.0000x reference)
"""Multi-head causal self-attention for TRN2, 8 NeuronCores.

Sharding: core i handles (batch b = i//2, head-group g = i%2); each head-group
is 8 of the 16 heads (local dims DL=512).  Computed in "transposed" space
(no on-device transposes).

Speed structure (v2):
  * fp8e4 DoubleRow matmuls (2 contraction rows/cycle) for the QKV
    projection, PV, and output projection of blocks j>=1; block j=0
    (queries+keys 0..511) stays bf16 because with few attended keys fp8
    quantization noise doesn't average out (rel-err budget).
  * Q^T/K^T stored head-pair packed ([128, T]: rows 0:64 = head 2i, rows
    64:128 = head 2i+1).  QK^T runs per-head with K=64 contraction; the
    two heads of a pair alternate PE row-strips so consecutive matmuls
    execute concurrently (row tiling) -> ~2x QK throughput.
  * V staged pre-scaled by 32 with a 32-valued tail column so the softmax
    row-sum rides the PV matmul ([V_e|32|V_o|32] fp8 chunk-pairs for
    DoubleRow).  exp uses bias -ln16 to keep e^s in fp8e4 range; all
    scales cancel in the normalize.  W_proj is staged *32 for the fp8
    path; the host divides rows >=512 by 32 at the end.
  * QKV of block j+1 and the projection of block j-1 interleave into
    attention block j so the PE never idles behind the ACT-bound softmax.
  * All dtype casts happen on the host (bf16/fp8 DMA); V staging and Q/K
    PSUM copy-outs are single strided DVE ops; reciprocals batched 8 heads
    at a time; mask-muls split DVE/GpSimd; projection copy-outs on GpSimd.
"""

import numpy as np
import ml_dtypes
from contextlib import ExitStack

import concourse.bass as bass
import concourse.mybir as mybir
import concourse.tile as tile
from concourse import bacc
from concourse.bass_utils import run_bass_kernel_spmd

B, T, D, H = 4, 2048, 1024, 16
DK = 64            # head dim
HL = 8             # heads per core
DL = HL * DK       # 512 local head dims per core
N_CORES = 8

F32 = mybir.dt.float32
F32R = mybir.dt.float32r
BF16 = mybir.dt.bfloat16
F8 = mybir.dt.float8e4
EXP = mybir.ActivationFunctionType.Exp
DR = mybir.MatmulPerfMode.DoubleRow
MULT = mybir.AluOpType.mult
ADD = mybir.AluOpType.add

E4NP = ml_dtypes.float8_e4m3
BFNP = ml_dtypes.bfloat16

TQ = 512           # query block size
NQB = T // TQ      # 4
NP = T // 256      # 8 key chunk-pairs
LN16 = float(np.log(16.0))

_CACHE = {}


def _build(causal: bool):
    nc = bacc.Bacc("TRN2", target_bir_lowering=False, debug=False,
                   num_devices=N_CORES)
    xbf_d = nc.dram_tensor("xbf", [D, TQ], BF16, kind="ExternalInput").ap()
    xf8_d = nc.dram_tensor("xf8", [4, 128, 2, T], F8,
                           kind="ExternalInput").ap()
    wqkbf_d = nc.dram_tensor("wqkbf", [D, 2 * DL], BF16,
                             kind="ExternalInput").ap()
    wqk8_d = nc.dram_tensor("wqk8", [4, 128, 2, 2 * DL], F8,
                            kind="ExternalInput").ap()
    wvbf_d = nc.dram_tensor("wvbf", [D, DL], BF16, kind="ExternalInput").ap()
    wv8_d = nc.dram_tensor("wv8", [4, 128, 2, DL], F8,
                           kind="ExternalInput").ap()
    wpbf_d = nc.dram_tensor("wpbf", [DL, D], BF16, kind="ExternalInput").ap()
    wp8_d = nc.dram_tensor("wp8", [2, 128, 2, D], F8,
                           kind="ExternalInput").ap()
    bqk_d = nc.dram_tensor("bqk", [8, 128, 1], F32, kind="ExternalInput").ap()
    bv_d = nc.dram_tensor("bv", [1, DL], F32, kind="ExternalInput").ap()
    masks_d = nc.dram_tensor("masks", [128, 4 * TQ], BF16,
                             kind="ExternalInput").ap()
    out_d = nc.dram_tensor("out", [T, D], F32, kind="ExternalOutput").ap()

    with tile.TileContext(nc) as tc, ExitStack() as top:
        persist = top.enter_context(tc.tile_pool(name="persist", bufs=1))

        # head-pair packed Q^T/K^T: rows 0:64 head 2i, rows 64:128 head 2i+1
        tileQ = [persist.tile([128, T], BF16, tag=f"tq{i}", name=f"tq{i}")
                 for i in range(4)]
        tileK = [persist.tile([128, T], BF16, tag=f"tk{i}", name=f"tk{i}")
                 for i in range(4)]
        # fp8 V chunk-pairs: (p, par, h, m) = [V_h | 32 | pad], *32-scaled
        vs2 = [persist.tile([128, 2, HL, 80], F8, tag=f"v2_{c}",
                            name=f"v2_{c}") for c in range(NP)]
        # bf16 V for block 0 (causal only): (p, h, m) = [V_h | 32]
        vs_bf = [persist.tile([128, HL, 65], BF16, tag=f"vb{t}",
                              name=f"vb{t}") for t in range(4)]
        yT_bf = [persist.tile([128, TQ], BF16, tag=f"yb{g}", name=f"yb{g}")
                 for g in range(4)]
        yTd = [persist.tile([128, 2, T], F8, tag=f"yd{g}",
                            name=f"yd{g}") for g in range(2)]
        wqk_bf = [persist.tile([128, 2 * DL], BF16, tag=f"wqb{d}",
                               name=f"wqb{d}") for d in range(8)]
        wqk8 = [persist.tile([128, 2, 2 * DL], F8, tag=f"wq8{g}",
                             name=f"wq8{g}") for g in range(4)]
        wv_bf = [persist.tile([128, DL], BF16, tag=f"wvb{d}",
                              name=f"wvb{d}") for d in range(8)]
        wv8 = [persist.tile([128, 2, DL], F8, tag=f"wv8{g}",
                            name=f"wv8{g}") for g in range(4)]
        wp_bf = [persist.tile([128, D], BF16, tag=f"wpb{g}",
                              name=f"wpb{g}") for g in range(4)]
        wp8 = [persist.tile([128, 2, D], F8, tag=f"wp8{g}",
                            name=f"wp8{g}") for g in range(2)]
        maskp = persist.tile([128, 4, TQ], BF16, tag="maskp", name="maskp")
        bqk_sb = [persist.tile([128, 1], F32, tag=f"bqk{m}", name=f"bqk{m}")
                  for m in range(8)]
        bv_r = persist.tile([1, DL], F32R, tag="bv_r", name="bv_r")
        ones_r = persist.tile([1, 128], F32R, tag="ones_r", name="ones_r")
        nln16 = persist.tile([128, 1], F32, tag="nln16", name="nln16")

        initp = top.enter_context(tc.tile_pool(name="initp", bufs=1))
        xstage = top.enter_context(tc.tile_pool(name="xstage", bufs=1))
        ppool = top.enter_context(tc.tile_pool(name="ppool", bufs=1))
        npool = top.enter_context(tc.tile_pool(name="npool", bufs=2))
        opool = top.enter_context(tc.tile_pool(name="opool", bufs=3))
        ps_a = top.enter_context(tc.tile_pool(name="ps_a", bufs=2,
                                              space="PSUM"))
        ps_s = top.enter_context(tc.tile_pool(name="ps_s", bufs=2,
                                              space="PSUM"))
        ps_o = top.enter_context(tc.tile_pool(name="ps_o", bufs=1,
                                              space="PSUM"))

        # ---------------- one-time init (j0-critical DMAs first) -----------
        for d in range(8):
            eng = nc.gpsimd if d % 2 == 0 else nc.sync
            eng.dma_start(wqk_bf[d][:],
                          wqkbf_d[d * 128:(d + 1) * 128, :])
        for d in range(8):
            nc.scalar.dma_start(wv_bf[d][:], wvbf_d[d * 128:(d + 1) * 128, :])
        for m in range(8):
            nc.gpsimd.dma_start(bqk_sb[m][:], bqk_d[m:m + 1, :, :])
        bv_f = initp.tile([1, DL], F32, tag="bv_f", name="bv_f")
        nc.gpsimd.dma_start(bv_f[:], bv_d)
        if causal:
            nc.gpsimd.dma_start(maskp[:, 0:4, :], masks_d)
        for g in range(4):
            nc.scalar.dma_start(wqk8[g][:], wqk8_d[g:g + 1, :, :, :])
            nc.scalar.dma_start(wv8[g][:], wv8_d[g:g + 1, :, :, :])
        for g in range(4):
            nc.gpsimd.dma_start(wp_bf[g][:], wpbf_d[g * 128:(g + 1) * 128, :])
        for g in range(2):
            nc.scalar.dma_start(wp8[g][:], wp8_d[g:g + 1, :, :, :])
        nc.vector.memset(nln16[:], -LN16)
        ones_f = initp.tile([1, 128], F32, tag="ones_f", name="ones_f")
        nc.vector.memset(ones_f[:], 1.0)
        nc.vector.tensor_copy(ones_r[:], ones_f[:])
        nc.vector.tensor_copy(bv_r[:], bv_f[:])
        for c in range(NP):
            nc.vector.memset(vs2[c][:, :, :, 64:65], 32.0)
        for t in range(4):
            nc.vector.memset(vs_bf[t][:, :, 64:65], 32.0)

        # ---------------- phase-1 step emitters ----------------
        def ph1_steps(j):
            jsl = slice(j * TQ, (j + 1) * TQ)
            steps = []
            bf = causal and j == 0
            if bf:
                xr = [xstage.tile([128, TQ], BF16, tag=f"xb{d}",
                                  name=f"xb{d}") for d in range(8)]

                def dma_x():
                    for d in range(8):
                        nc.sync.dma_start(xr[d][:],
                                          xbf_d[d * 128:(d + 1) * 128, :])
            else:
                xr = [xstage.tile([128, 2, TQ], F8, tag=f"x8{g}", bufs=2,
                                  name=f"x8{g}_{j}") for g in range(4)]

                def dma_x():
                    for g in range(4):
                        nc.sync.dma_start(xr[g][:], xf8_d[g:g + 1, :, :, jsl])
            steps.append(dma_x)

            def qk_tile(m):
                def emit():
                    ps = ps_a.tile([128, TQ], F32, tag="ps",
                                   name=f"psqk{j}_{m}")
                    if bf:
                        for d in range(8):
                            nc.tensor.matmul(
                                ps[:], wqk_bf[d][:, m * 128:(m + 1) * 128],
                                xr[d][:], start=(d == 0), stop=(d == 7))
                    else:
                        for g in range(4):
                            nc.tensor.matmul(
                                ps[:], wqk8[g][:, :, m * 128:(m + 1) * 128],
                                xr[g][:], start=(g == 0), stop=(g == 3),
                                perf_mode=DR)
                    dst = tileQ[m] if m < 4 else tileK[m - 4]
                    if bf:
                        nc.vector.tensor_scalar(
                            dst[:, jsl], ps[:], bqk_sb[m][:], None, op0=ADD)
                    else:
                        nc.vector.tensor_scalar(
                            dst[:, jsl], ps[:], 1.0 / 32.0, bqk_sb[m][:],
                            op0=MULT, op1=ADD)
                return emit

            for m in range(8):
                steps.append(qk_tile(m))

            def v_tile(tt):
                def emit():
                    c = tt % 4
                    ps = ps_a.tile([128, HL, 64], F32, tag="ps",
                                   name=f"psv{tt}")
                    if bf:
                        for d in range(8):
                            nc.tensor.matmul(
                                ps[:], xr[d][:, c * 128:(c + 1) * 128],
                                wv_bf[d][:], start=(d == 0), stop=False)
                    else:
                        for g in range(4):
                            nc.tensor.matmul(
                                ps[:], xr[g][:, :, c * 128:(c + 1) * 128],
                                wv8[g][:], start=(g == 0), stop=False,
                                perf_mode=DR)
                    nc.tensor.matmul(ps[:], ones_r[:, 0:128], bv_r[:],
                                     start=False, stop=True)
                    par = tt % 2
                    nc.vector.tensor_copy(
                        vs2[tt // 2][:, par:par + 1, :, 0:64], ps[:])
                    if causal and tt < 4:
                        nc.vector.tensor_copy(vs_bf[tt][:, :, 0:64], ps[:])
                return emit

            for tt in range(4 * j, 4 * j + 4):
                steps.append(v_tile(tt))
            return steps

        # ---------------- output projection steps ----------------
        def proj_step(t, nb):
            def emit():
                nsl = slice(nb * 512, (nb + 1) * 512)
                ps = ps_a.tile([128, TQ], F32, tag="ps", name=f"ps3_{t}_{nb}")
                if causal and t < 4:
                    for g in range(4):
                        nc.tensor.matmul(
                            ps[:], yT_bf[g][:, t * 128:(t + 1) * 128],
                            wp_bf[g][:, nsl], start=(g == 0), stop=(g == 3))
                else:
                    toff = t * 128
                    for g in range(2):
                        nc.tensor.matmul(
                            ps[:], yTd[g][:, :, toff:toff + 128],
                            wp8[g][:, :, nsl], start=(g == 0), stop=(g == 1),
                            perf_mode=DR)
                ot = opool.tile([128, TQ], F32, tag="ot", name=f"ot{t}_{nb}")
                nc.vector.tensor_copy(ot[:], ps[:])
                nc.sync.dma_start(out_d[t * 128:(t + 1) * 128, nsl], ot[:])
            return emit

        # ---------------- main fused loop ----------------
        for fn in ph1_steps(0):
            fn()
        pending = []
        for j in range(NQB):
            jsl = slice(j * TQ, (j + 1) * TQ)
            bf = causal and j == 0
            npairs = 2 * (j + 1) if causal else NP
            newsteps = ph1_steps(j + 1) if j + 1 < NQB else []
            pj = {0: [], 1: [], 2: [0], 3: [1, 2]}[j]
            projs = [proj_step(t, nb) for jj in pj
                     for t in range(4 * jj, 4 * jj + 4) for nb in range(2)]
            mixed = []
            while newsteps or projs:
                if newsteps:
                    mixed.append(newsteps.pop(0))
                if projs:
                    mixed.append(projs.pop(0))
            pending += mixed
            niter = 4 * npairs
            rate = len(pending) / max(1, niter)
            acc = 0.0

            for i in range(4):
                hA, hB = 2 * i, 2 * i + 1
                poA = ps_o.tile([128, TQ], F32, tag="poA", name=f"poA{j}_{i}")
                poB = ps_o.tile([128, TQ], F32, tag="poB", name=f"poB{j}_{i}")
                pends = {hA: None, hB: None}
                for ci in range(npairs):
                    ke = slice(ci * 256, ci * 256 + 128)
                    ko = slice(ci * 256 + 128, ci * 256 + 256)
                    diag = causal and ci >= 2 * j
                    r = ci - 2 * j
                    # fully-masked query-column prefixes on diagonal chunks
                    sk_e = 128 * 2 * r if diag else 0
                    sk_o = 128 * (2 * r + 1) if diag else 0
                    je = slice(j * TQ + sk_e, (j + 1) * TQ)
                    jo = slice(j * TQ + sk_o, (j + 1) * TQ)
                    ssA = ps_s.tile([128, 2, TQ], F32, tag="ss",
                                    name=f"ssA{j}_{i}_{ci}")
                    ssB = ps_s.tile([128, 2, TQ], F32, tag="ss",
                                    name=f"ssB{j}_{i}_{ci}")
                    nc.tensor.matmul(ssA[:, 0:1, sk_e:], tileK[i][0:64, ke],
                                     tileQ[i][0:64, je],
                                     start=True, stop=True)
                    nc.tensor.matmul(ssB[:, 0:1, sk_e:], tileK[i][64:128, ke],
                                     tileQ[i][64:128, je],
                                     start=True, stop=True)
                    nc.tensor.matmul(ssA[:, 1:2, sk_o:], tileK[i][0:64, ko],
                                     tileQ[i][0:64, jo],
                                     start=True, stop=True)
                    nc.tensor.matmul(ssB[:, 1:2, sk_o:], tileK[i][64:128, ko],
                                     tileQ[i][64:128, jo],
                                     start=True, stop=True)
                    kind = (BF16 if bf else F8)
                    ptA = ppool.tile([128, 2, TQ], kind,
                                     tag="ptb" if bf else "pt8",
                                     bufs=4 if bf else 8,
                                     name=f"ptA{j}_{i}_{ci}")
                    ptB = ppool.tile([128, 2, TQ], kind,
                                     tag="ptb" if bf else "pt8",
                                     bufs=4 if bf else 8,
                                     name=f"ptB{j}_{i}_{ci}")
                    acc += rate / 2
                    while acc >= 1.0 and pending:
                        pending.pop(0)()
                        acc -= 1.0
                    po_ = 256 * r if diag else 0
                    nc.scalar.activation(ptA[:, 0:2, po_:],
                                         ssA[:, 0:2, po_:],
                                         EXP, bias=nln16[:], scale=0.125)
                    nc.scalar.activation(ptB[:, 0:2, po_:],
                                         ssB[:, 0:2, po_:],
                                         EXP, bias=nln16[:], scale=0.125)
                    if diag:
                        ce_, co_ = 128 * (2 * r + 1), 128 * (2 * r + 2)
                        for pt in (ptA, ptB):
                            nc.vector.tensor_mul(pt[:, 0:1, po_:ce_],
                                                 pt[:, 0:1, po_:ce_],
                                                 maskp[:, 2 * r:2 * r + 1,
                                                       po_:ce_])
                            nc.vector.tensor_mul(pt[:, 1:2, po_:co_],
                                                 pt[:, 1:2, po_:co_],
                                                 maskp[:, 2 * r + 1:
                                                       2 * r + 2, po_:co_])
                    st = (ci == 1)
                    for po, h in ((poA, hA), (poB, hB)):
                        pend = pends[h]
                        if pend is None:
                            continue
                        pc, ppt = pend
                        if bf:
                            for e in range(2):
                                sk = 128 * (2 * pc + e) if causal else 0
                                nc.tensor.matmul(
                                    po[0:65, sk:],
                                    vs_bf[2 * pc + e][:, h:h + 1, :],
                                    ppt[:, e:e + 1, sk:],
                                    start=(st and e == 0), stop=False)
                        else:
                            pk = 256 if (causal and pc == 2 * j + 1) else 0
                            nc.tensor.matmul(
                                po[0:65, pk:],
                                vs2[pc][:, :, h:h + 1, 0:65],
                                ppt[:, 0:2, pk:], start=st, stop=False,
                                perf_mode=DR)
                    acc += rate / 2
                    while acc >= 1.0 and pending:
                        pending.pop(0)()
                        acc -= 1.0
                    pends[hA] = (ci, ptA)
                    pends[hB] = (ci, ptB)
                one = (npairs == 1)
                for po, h in ((poA, hA), (poB, hB)):
                    pc, ppt = pends[h]
                    if bf:
                        for e in range(2):
                            sk = 128 * (2 * pc + e) if causal else 0
                            nc.tensor.matmul(
                                po[0:65, sk:],
                                vs_bf[2 * pc + e][:, h:h + 1, :],
                                ppt[:, e:e + 1, sk:],
                                start=(one and e == 0), stop=(e == 1))
                    else:
                        pk = 256 if (causal and pc == 2 * j + 1) else 0
                        nc.tensor.matmul(
                            po[0:65, pk:],
                            vs2[pc][:, :, h:h + 1, 0:65],
                            ppt[:, 0:2, pk:], start=one, stop=True,
                            perf_mode=DR)
                    sums = npool.tile([1, TQ], F32, tag="sums",
                                      name=f"sm{j}_{h}", bufs=3)
                    nc.vector.tensor_copy(sums[:], po[64:65, :])
                    o_sb = npool.tile([64, TQ], BF16, tag="osb",
                                      name=f"ob{j}_{h}", bufs=3)
                    nc.vector.tensor_copy(o_sb[:], po[0:64, :])
                    recip = npool.tile([1, TQ], F32, tag="recip",
                                       name=f"rc{j}_{h}", bufs=3)
                    nc.vector.reciprocal_approx_fast(out=recip[:],
                                                     in_=sums[:])
                    recip_r = npool.tile([1, TQ], F32R, tag="recipr",
                                         name=f"rr{j}_{h}", bufs=3)
                    nc.vector.tensor_copy(recip_r[:], recip[:])
                    pb = ps_a.tile([64, TQ], F32, tag="ps",
                                   name=f"pb{j}_{h}")
                    nc.tensor.matmul(pb[:], ones_r[:, 0:64], recip_r[:],
                                     start=True, stop=True)
                    if bf:
                        nc.vector.tensor_mul(
                            yT_bf[h // 2][(h % 2) * 64:(h % 2) * 64 + 64, :],
                            o_sb[:], pb[:])
                    else:
                        gg, e = h // 4, (h // 2) % 2
                        nc.vector.tensor_mul(
                            yTd[gg][(h % 2) * 64:(h % 2) * 64 + 64,
                                    e:e + 1, jsl],
                            o_sb[:], pb[:])

            while pending:
                pending.pop(0)()
        for t in range(4 * (NQB - 1), 4 * NQB):
            for nb in range(2):
                proj_step(t, nb)()

    nc.compile()
    return nc


def _get_nc(causal: bool):
    if causal not in _CACHE:
        _CACHE[causal] = _build(causal)
    return _CACHE[causal]


def _host_masks() -> np.ndarray:
    i = np.arange(128)[:, None]
    jj = np.arange(TQ)[None, :]
    blocks = [(jj >= i + s * 128).astype(np.float32) for s in range(4)]
    return np.ascontiguousarray(
        np.concatenate(blocks, axis=1).astype(BFNP))


def _pair4(a, width):
    """[1024, width] -> [4, 128, 2, width] DoubleRow layout."""
    return np.ascontiguousarray(
        a.reshape(4, 2, 128, width).transpose(0, 2, 1, 3))


def _make_in_maps(x, W_qkv, b_qkv, W_proj):
    masks_np = _host_masks()
    in_maps = []
    for core in range(N_CORES):
        b, g = core // 2, core % 2
        qc = slice(g * DL, (g + 1) * DL)
        kc = slice(D + g * DL, D + (g + 1) * DL)
        vc = slice(2 * D + g * DL, 2 * D + (g + 1) * DL)
        Wq = np.concatenate([W_qkv[:, qc], W_qkv[:, kc]], axis=1)  # [D,1024]
        Wv32 = W_qkv[:, vc] * np.float32(32.0)                     # [D, 512]
        Wp = W_proj[g * DL:(g + 1) * DL, :]                        # [512,1024]
        xT = np.ascontiguousarray(x[b].T)                          # [D, T]
        in_maps.append({
            "xbf": np.ascontiguousarray(xT[:, 0:TQ].astype(BFNP)),
            "xf8": _pair4(xT.astype(E4NP), T),
            "wqkbf": np.ascontiguousarray(Wq.astype(BFNP)),
            "wqk8": _pair4((Wq * np.float32(32.0)).astype(E4NP), 2 * DL),
            "wvbf": np.ascontiguousarray(Wv32.astype(BFNP)),
            "wv8": _pair4(Wv32.astype(E4NP), DL),
            "wpbf": np.ascontiguousarray(Wp.astype(BFNP)),
            "wp8": np.ascontiguousarray(
                (Wp * np.float32(32.0)).astype(E4NP)
                .reshape(2, 2, 128, D).transpose(0, 2, 1, 3)),
            "bqk": np.ascontiguousarray(
                np.concatenate([b_qkv[qc], b_qkv[kc]]).reshape(8, 128, 1)),
            "bv": np.ascontiguousarray(
                (b_qkv[vc] * np.float32(32.0)).reshape(1, DL)),
            "masks": masks_np,
        })
    return in_maps


def kernel(x, mask, W_qkv, b_qkv, W_proj, b_proj):
    x = np.asarray(x, dtype=np.float32)
    mask2d = np.asarray(mask, dtype=np.int32).reshape(T, T)
    W_qkv = np.asarray(W_qkv, dtype=np.float32)
    b_qkv = np.asarray(b_qkv, dtype=np.float32)
    W_proj = np.asarray(W_proj, dtype=np.float32)
    b_proj = np.asarray(b_proj, dtype=np.float32)

    if np.array_equal(mask2d, np.tril(np.ones((T, T), dtype=np.int32))):
        causal = True
    elif np.all(mask2d == 1):
        causal = False
    else:
        raise NotImplementedError("only causal (tril) or all-ones masks")

    nc = _get_nc(causal)
    in_maps = _make_in_maps(x, W_qkv, b_qkv, W_proj)
    res = run_bass_kernel_spmd(nc, in_maps, core_ids=list(range(N_CORES)))
    inv32 = np.float32(1.0 / 32.0)
    out = np.empty((B, T, D), dtype=np.float32)
    for b in range(B):
        acc = res.results[2 * b]["out"] + res.results[2 * b + 1]["out"]
        if causal:
            acc[TQ:] *= inv32
        else:
            acc *= inv32
        out[b] = acc + b_proj[None, :]
    return out


# revision 41
# speedup vs baseline: 1.0382x; 1.0382x over previous
"""Multi-head causal self-attention for TRN2, 8 NeuronCores.

Sharding: core i handles (batch b = i//2, head-group g = i%2); each head-group
is 8 of the 16 heads (local dims DL=512).  Computed in "transposed" space
(no on-device transposes).

Speed structure (v2):
  * fp8e4 DoubleRow matmuls (2 contraction rows/cycle) for the QKV
    projection, PV, and output projection of blocks j>=1; block j=0
    (queries+keys 0..511) stays bf16 because with few attended keys fp8
    quantization noise doesn't average out (rel-err budget).
  * Q^T/K^T stored head-pair packed ([128, T]: rows 0:64 = head 2i, rows
    64:128 = head 2i+1).  QK^T runs per-head with K=64 contraction; the
    two heads of a pair alternate PE row-strips so consecutive matmuls
    execute concurrently (row tiling) -> ~2x QK throughput.
  * V staged pre-scaled by 32 with a 32-valued tail column so the softmax
    row-sum rides the PV matmul ([V_e|32|V_o|32] fp8 chunk-pairs for
    DoubleRow).  exp uses bias -ln16 to keep e^s in fp8e4 range; all
    scales cancel in the normalize.  W_proj is staged *32 for the fp8
    path; the host divides rows >=512 by 32 at the end.
  * QKV of block j+1 and the projection of block j-1 interleave into
    attention block j so the PE never idles behind the ACT-bound softmax.
  * All dtype casts happen on the host (bf16/fp8 DMA); V staging and Q/K
    PSUM copy-outs are single strided DVE ops; reciprocals batched 8 heads
    at a time; mask-muls split DVE/GpSimd; projection copy-outs on GpSimd.
"""

import numpy as np
import ml_dtypes
from contextlib import ExitStack

import concourse.bass as bass
import concourse.mybir as mybir
import concourse.tile as tile
from concourse import bacc
from concourse.bass_utils import run_bass_kernel_spmd

B, T, D, H = 4, 2048, 1024, 16
DK = 64            # head dim
HL = 8             # heads per core
DL = HL * DK       # 512 local head dims per core
N_CORES = 8

F32 = mybir.dt.float32
F32R = mybir.dt.float32r
BF16 = mybir.dt.bfloat16
F8 = mybir.dt.float8e4
EXP = mybir.ActivationFunctionType.Exp
DR = mybir.MatmulPerfMode.DoubleRow
MULT = mybir.AluOpType.mult
ADD = mybir.AluOpType.add

E4NP = ml_dtypes.float8_e4m3
BFNP = ml_dtypes.bfloat16

TQ = 512           # query block size
NQB = T // TQ      # 4
NP = T // 256      # 8 key chunk-pairs
LN16 = float(np.log(16.0))

_CACHE = {}


def _build(causal: bool):
    nc = bacc.Bacc("TRN2", target_bir_lowering=False, debug=False,
                   num_devices=N_CORES)
    xbf_d = nc.dram_tensor("xbf", [D, TQ], BF16, kind="ExternalInput").ap()
    xf8_d = nc.dram_tensor("xf8", [4, 128, 2, T], F8,
                           kind="ExternalInput").ap()
    wqkbf_d = nc.dram_tensor("wqkbf", [D, 2 * DL], BF16,
                             kind="ExternalInput").ap()
    wqk8_d = nc.dram_tensor("wqk8", [4, 128, 2, 2 * DL], F8,
                            kind="ExternalInput").ap()
    wvbf_d = nc.dram_tensor("wvbf", [D, DL], BF16, kind="ExternalInput").ap()
    wv8_d = nc.dram_tensor("wv8", [4, 128, 2, DL], F8,
                           kind="ExternalInput").ap()
    wpbf_d = nc.dram_tensor("wpbf", [DL, D], BF16, kind="ExternalInput").ap()
    wp8_d = nc.dram_tensor("wp8", [2, 128, 2, D], F8,
                           kind="ExternalInput").ap()
    bqk_d = nc.dram_tensor("bqk", [8, 128, 1], F32, kind="ExternalInput").ap()
    bv_d = nc.dram_tensor("bv", [1, DL], F32, kind="ExternalInput").ap()
    masks_d = nc.dram_tensor("masks", [128, 4 * TQ], BF16,
                             kind="ExternalInput").ap()
    out_d = nc.dram_tensor("out", [T, D], F32, kind="ExternalOutput").ap()

    with tile.TileContext(nc) as tc, ExitStack() as top:
        persist = top.enter_context(tc.tile_pool(name="persist", bufs=1))

        # head-pair packed Q^T/K^T: rows 0:64 head 2i, rows 64:128 head 2i+1
        tileQ = [persist.tile([128, T], BF16, tag=f"tq{i}", name=f"tq{i}")
                 for i in range(4)]
        tileK = [persist.tile([128, T], BF16, tag=f"tk{i}", name=f"tk{i}")
                 for i in range(4)]
        # fp8 V chunk-pairs: (p, par, h, m) = [V_h | 32 | pad], *32-scaled
        vs2 = [persist.tile([128, 2, HL, 80], F8, tag=f"v2_{c}",
                            name=f"v2_{c}") for c in range(NP)]
        # bf16 V for block 0 (causal only): (p, h, m) = [V_h | 32]
        vs_bf = [persist.tile([128, HL, 65], BF16, tag=f"vb{t}",
                              name=f"vb{t}") for t in range(4)]
        yT_bf = [persist.tile([128, TQ], BF16, tag=f"yb{g}", name=f"yb{g}")
                 for g in range(4)]
        yTd = [persist.tile([128, 2, T], F8, tag=f"yd{g}",
                            name=f"yd{g}") for g in range(2)]
        wqk_bf = [persist.tile([128, 2 * DL], BF16, tag=f"wqb{d}",
                               name=f"wqb{d}") for d in range(8)]
        wqk8 = [persist.tile([128, 2, 2 * DL], F8, tag=f"wq8{g}",
                             name=f"wq8{g}") for g in range(4)]
        wv_bf = [persist.tile([128, DL], BF16, tag=f"wvb{d}",
                              name=f"wvb{d}") for d in range(8)]
        wv8 = [persist.tile([128, 2, DL], F8, tag=f"wv8{g}",
                            name=f"wv8{g}") for g in range(4)]
        wp_bf = [persist.tile([128, D], BF16, tag=f"wpb{g}",
                              name=f"wpb{g}") for g in range(4)]
        wp8 = [persist.tile([128, 2, D], F8, tag=f"wp8{g}",
                            name=f"wp8{g}") for g in range(2)]
        maskp = persist.tile([128, 4, TQ], BF16, tag="maskp", name="maskp")
        bqk_sb = [persist.tile([128, 1], F32, tag=f"bqk{m}", name=f"bqk{m}")
                  for m in range(8)]
        bv_r = persist.tile([1, DL], F32R, tag="bv_r", name="bv_r")
        ones_r = persist.tile([1, 128], F32R, tag="ones_r", name="ones_r")
        nln16 = persist.tile([128, 1], F32, tag="nln16", name="nln16")

        initp = top.enter_context(tc.tile_pool(name="initp", bufs=1))
        xstage = top.enter_context(tc.tile_pool(name="xstage", bufs=1))
        ppool = top.enter_context(tc.tile_pool(name="ppool", bufs=1))
        npool = top.enter_context(tc.tile_pool(name="npool", bufs=2))
        opool = top.enter_context(tc.tile_pool(name="opool", bufs=3))
        ps_a = top.enter_context(tc.tile_pool(name="ps_a", bufs=2,
                                              space="PSUM"))
        ps_s = top.enter_context(tc.tile_pool(name="ps_s", bufs=2,
                                              space="PSUM"))
        ps_o = top.enter_context(tc.tile_pool(name="ps_o", bufs=1,
                                              space="PSUM"))

        # ---------------- one-time init (j0-critical DMAs first) -----------
        for d in range(8):
            eng = nc.gpsimd if d % 2 == 0 else nc.sync
            eng.dma_start(wqk_bf[d][:],
                          wqkbf_d[d * 128:(d + 1) * 128, :])
        for d in range(8):
            nc.scalar.dma_start(wv_bf[d][:], wvbf_d[d * 128:(d + 1) * 128, :])
        for m in range(8):
            nc.gpsimd.dma_start(bqk_sb[m][:], bqk_d[m:m + 1, :, :])
        bv_f = initp.tile([1, DL], F32, tag="bv_f", name="bv_f")
        nc.gpsimd.dma_start(bv_f[:], bv_d)
        if causal:
            nc.gpsimd.dma_start(maskp[:, 0:4, :], masks_d)
        for g in range(4):
            nc.scalar.dma_start(wqk8[g][:], wqk8_d[g:g + 1, :, :, :])
            nc.scalar.dma_start(wv8[g][:], wv8_d[g:g + 1, :, :, :])
        for g in range(4):
            nc.gpsimd.dma_start(wp_bf[g][:], wpbf_d[g * 128:(g + 1) * 128, :])
        for g in range(2):
            nc.scalar.dma_start(wp8[g][:], wp8_d[g:g + 1, :, :, :])
        nc.vector.memset(nln16[:], -LN16)
        ones_f = initp.tile([1, 128], F32, tag="ones_f", name="ones_f")
        nc.vector.memset(ones_f[:], 1.0)
        nc.vector.tensor_copy(ones_r[:], ones_f[:])
        nc.vector.tensor_copy(bv_r[:], bv_f[:])
        for c in range(NP):
            nc.vector.memset(vs2[c][:, :, :, 64:65], 32.0)
        for t in range(4):
            nc.vector.memset(vs_bf[t][:, :, 64:65], 32.0)

        # ---------------- phase-1 step emitters ----------------
        def ph1_steps(j):
            jsl = slice(j * TQ, (j + 1) * TQ)
            steps = []
            bf = causal and j == 0
            if bf:
                xr = [xstage.tile([128, TQ], BF16, tag=f"xb{d}",
                                  name=f"xb{d}") for d in range(8)]

                def dma_x():
                    for d in range(8):
                        nc.sync.dma_start(xr[d][:],
                                          xbf_d[d * 128:(d + 1) * 128, :])
            else:
                xr = [xstage.tile([128, 2, TQ], F8, tag=f"x8{g}", bufs=2,
                                  name=f"x8{g}_{j}") for g in range(4)]

                def dma_x():
                    for g in range(4):
                        nc.sync.dma_start(xr[g][:], xf8_d[g:g + 1, :, :, jsl])
            steps.append(dma_x)

            def qk_tile(m):
                def emit():
                    ps = ps_a.tile([128, TQ], F32, tag="ps",
                                   name=f"psqk{j}_{m}")
                    if bf:
                        for d in range(8):
                            nc.tensor.matmul(
                                ps[:], wqk_bf[d][:, m * 128:(m + 1) * 128],
                                xr[d][:], start=(d == 0), stop=(d == 7))
                    else:
                        for g in range(4):
                            nc.tensor.matmul(
                                ps[:], wqk8[g][:, :, m * 128:(m + 1) * 128],
                                xr[g][:], start=(g == 0), stop=(g == 3),
                                perf_mode=DR)
                    dst = tileQ[m] if m < 4 else tileK[m - 4]
                    if bf:
                        nc.vector.tensor_scalar(
                            dst[:, jsl], ps[:], bqk_sb[m][:], None, op0=ADD)
                    else:
                        nc.vector.tensor_scalar(
                            dst[:, jsl], ps[:], 1.0 / 32.0, bqk_sb[m][:],
                            op0=MULT, op1=ADD)
                return emit

            for m in range(8):
                steps.append(qk_tile(m))

            def v_tile(tt):
                def emit():
                    c = tt % 4
                    ps = ps_a.tile([128, HL, 64], F32, tag="ps",
                                   name=f"psv{tt}")
                    if bf:
                        for d in range(8):
                            nc.tensor.matmul(
                                ps[:], xr[d][:, c * 128:(c + 1) * 128],
                                wv_bf[d][:], start=(d == 0), stop=False)
                    else:
                        for g in range(4):
                            nc.tensor.matmul(
                                ps[:], xr[g][:, :, c * 128:(c + 1) * 128],
                                wv8[g][:], start=(g == 0), stop=False,
                                perf_mode=DR)
                    nc.tensor.matmul(ps[:], ones_r[:, 0:128], bv_r[:],
                                     start=False, stop=True)
                    par = tt % 2
                    nc.vector.tensor_copy(
                        vs2[tt // 2][:, par:par + 1, :, 0:64], ps[:])
                    if causal and tt < 4:
                        nc.vector.tensor_copy(vs_bf[tt][:, :, 0:64], ps[:])
                return emit

            for tt in range(4 * j, 4 * j + 4):
                steps.append(v_tile(tt))
            return steps

        # ---------------- output projection steps ----------------
        def proj_step(t, nb):
            def emit():
                nsl = slice(nb * 512, (nb + 1) * 512)
                ps = ps_a.tile([128, TQ], F32, tag="ps", name=f"ps3_{t}_{nb}")
                if causal and t < 4:
                    for g in range(4):
                        nc.tensor.matmul(
                            ps[:], yT_bf[g][:, t * 128:(t + 1) * 128],
                            wp_bf[g][:, nsl], start=(g == 0), stop=(g == 3))
                else:
                    toff = t * 128
                    for g in range(2):
                        nc.tensor.matmul(
                            ps[:], yTd[g][:, :, toff:toff + 128],
                            wp8[g][:, :, nsl], start=(g == 0), stop=(g == 1),
                            perf_mode=DR)
                ot = opool.tile([128, TQ], F32, tag="ot", name=f"ot{t}_{nb}")
                nc.vector.tensor_copy(ot[:], ps[:])
                nc.sync.dma_start(out_d[t * 128:(t + 1) * 128, nsl], ot[:])
            return emit

        # ---------------- main fused loop ----------------
        for fn in ph1_steps(0):
            fn()
        pending = []
        for j in range(NQB):
            jsl = slice(j * TQ, (j + 1) * TQ)
            bf = causal and j == 0
            npairs = 2 * (j + 1) if causal else NP
            newsteps = ph1_steps(j + 1) if j + 1 < NQB else []
            pj = {0: [], 1: [], 2: [0], 3: [1, 2]}[j]
            projs = [proj_step(t, nb) for jj in pj
                     for t in range(4 * jj, 4 * jj + 4) for nb in range(2)]
            mixed = []
            while newsteps or projs:
                if newsteps:
                    mixed.append(newsteps.pop(0))
                if projs:
                    mixed.append(projs.pop(0))
            pending += mixed
            niter = 4 * npairs
            rate = len(pending) / max(1, niter)
            acc = 0.0

            for i in range(4):
                hA, hB = 2 * i, 2 * i + 1
                poA = ps_o.tile([128, TQ], F32, tag="poA", name=f"poA{j}_{i}")
                poB = ps_o.tile([128, TQ], F32, tag="poB", name=f"poB{j}_{i}")
                pends = {hA: None, hB: None}
                for ci in range(npairs):
                    ke = slice(ci * 256, ci * 256 + 128)
                    ko = slice(ci * 256 + 128, ci * 256 + 256)
                    diag = causal and ci >= 2 * j
                    r = ci - 2 * j
                    # fully-masked query-column prefixes on diagonal chunks
                    sk_e = 128 * 2 * r if diag else 0
                    sk_o = 128 * (2 * r + 1) if diag else 0
                    je = slice(j * TQ + sk_e, (j + 1) * TQ)
                    jo = slice(j * TQ + sk_o, (j + 1) * TQ)
                    ssA = ps_s.tile([128, 2, TQ], F32, tag="ss",
                                    name=f"ssA{j}_{i}_{ci}")
                    ssB = ps_s.tile([128, 2, TQ], F32, tag="ss",
                                    name=f"ssB{j}_{i}_{ci}")
                    nc.tensor.matmul(ssA[:, 0:1, sk_e:], tileK[i][0:64, ke],
                                     tileQ[i][0:64, je],
                                     start=True, stop=True)
                    nc.tensor.matmul(ssB[:, 0:1, sk_e:], tileK[i][64:128, ke],
                                     tileQ[i][64:128, je],
                                     start=True, stop=True)
                    nc.tensor.matmul(ssA[:, 1:2, sk_o:], tileK[i][0:64, ko],
                                     tileQ[i][0:64, jo],
                                     start=True, stop=True)
                    nc.tensor.matmul(ssB[:, 1:2, sk_o:], tileK[i][64:128, ko],
                                     tileQ[i][64:128, jo],
                                     start=True, stop=True)
                    kind = (BF16 if bf else F8)
                    ptA = ppool.tile([128, 2, TQ], kind,
                                     tag="ptb" if bf else "pt8",
                                     bufs=4 if bf else 8,
                                     name=f"ptA{j}_{i}_{ci}")
                    ptB = ppool.tile([128, 2, TQ], kind,
                                     tag="ptb" if bf else "pt8",
                                     bufs=4 if bf else 8,
                                     name=f"ptB{j}_{i}_{ci}")
                    acc += rate / 2
                    while acc >= 1.0 and pending:
                        pending.pop(0)()
                        acc -= 1.0
                    po_ = 256 * r if diag else 0
                    nc.scalar.activation(ptA[:, 0:2, po_:],
                                         ssA[:, 0:2, po_:],
                                         EXP, bias=nln16[:], scale=0.125)
                    nc.scalar.activation(ptB[:, 0:2, po_:],
                                         ssB[:, 0:2, po_:],
                                         EXP, bias=nln16[:], scale=0.125)
                    if diag:
                        ce_, co_ = 128 * (2 * r + 1), 128 * (2 * r + 2)
                        for pt in (ptA, ptB):
                            nc.vector.tensor_mul(pt[:, 0:1, po_:ce_],
                                                 pt[:, 0:1, po_:ce_],
                                                 maskp[:, 2 * r:2 * r + 1,
                                                       po_:ce_])
                            nc.vector.tensor_mul(pt[:, 1:2, po_:co_],
                                                 pt[:, 1:2, po_:co_],
                                                 maskp[:, 2 * r + 1:
                                                       2 * r + 2, po_:co_])
                    st = (ci == 1)
                    for po, h in ((poA, hA), (poB, hB)):
                        pend = pends[h]
                        if pend is None:
                            continue
                        pc, ppt = pend
                        if bf:
                            for e in range(2):
                                sk = 128 * (2 * pc + e) if causal else 0
                                nc.tensor.matmul(
                                    po[0:65, sk:],
                                    vs_bf[2 * pc + e][:, h:h + 1, :],
                                    ppt[:, e:e + 1, sk:],
                                    start=(st and e == 0), stop=False)
                        else:
                            pk = 256 if (causal and pc == 2 * j + 1) else 0
                            nc.tensor.matmul(
                                po[0:65, pk:],
                                vs2[pc][:, :, h:h + 1, 0:65],
                                ppt[:, 0:2, pk:], start=st, stop=False,
                                perf_mode=DR)
                    acc += rate / 2
                    while acc >= 1.0 and pending:
                        pending.pop(0)()
                        acc -= 1.0
                    pends[hA] = (ci, ptA)
                    pends[hB] = (ci, ptB)
                one = (npairs == 1)
                for po, h in ((poA, hA), (poB, hB)):
                    pc, ppt = pends[h]
                    if bf:
                        for e in range(2):
                            sk = 128 * (2 * pc + e) if causal else 0
                            nc.tensor.matmul(
                                po[0:65, sk:],
                                vs_bf[2 * pc + e][:, h:h + 1, :],
                                ppt[:, e:e + 1, sk:],
                                start=(one and e == 0), stop=(e == 1))
                    else:
                        pk = 256 if (causal and pc == 2 * j + 1) else 0
                        nc.tensor.matmul(
                            po[0:65, pk:],
                            vs2[pc][:, :, h:h + 1, 0:65],
                            ppt[:, 0:2, pk:], start=one, stop=True,
                            perf_mode=DR)
                    sums = npool.tile([1, TQ], F32, tag="sums",
                                      name=f"sm{j}_{h}", bufs=3)
                    nc.vector.tensor_copy(sums[:], po[64:65, :])
                    o_sb = npool.tile([64, TQ], BF16, tag="osb",
                                      name=f"ob{j}_{h}", bufs=3)
                    nc.vector.tensor_copy(o_sb[:], po[0:64, :])
                    recip = npool.tile([1, TQ], F32, tag="recip",
                                       name=f"rc{j}_{h}", bufs=3)
                    nc.vector.reciprocal_approx_fast(out=recip[:],
                                                     in_=sums[:])
                    recip_r = npool.tile([1, TQ], F32R, tag="recipr",
                                         name=f"rr{j}_{h}", bufs=3)
                    nc.vector.tensor_copy(recip_r[:], recip[:])
                    pb = ps_a.tile([64, TQ], F32, tag="ps",
                                   name=f"pb{j}_{h}")
                    nc.tensor.matmul(pb[:], ones_r[:, 0:64], recip_r[:],
                                     start=True, stop=True)
                    if bf:
                        nc.vector.tensor_mul(
                            yT_bf[h // 2][(h % 2) * 64:(h % 2) * 64 + 64, :],
                            o_sb[:], pb[:])
                    else:
                        gg, e = h // 4, (h // 2) % 2
                        nc.vector.tensor_mul(
                            yTd[gg][(h % 2) * 64:(h % 2) * 64 + 64,
                                    e:e + 1, jsl],
                            o_sb[:], pb[:])

            while pending:
                pending.pop(0)()
        for t in range(4 * (NQB - 1), 4 * NQB):
            for nb in range(2):
                proj_step(t, nb)()

    nc.compile()
    return nc


def _get_nc(causal: bool):
    if causal not in _CACHE:
        _CACHE[causal] = _build(causal)
    return _CACHE[causal]


def _host_masks() -> np.ndarray:
    i = np.arange(128)[:, None]
    jj = np.arange(TQ)[None, :]
    blocks = [(jj >= i + s * 128).astype(np.float32) for s in range(4)]
    return np.ascontiguousarray(
        np.concatenate(blocks, axis=1).astype(BFNP))


def _pair4(a, width):
    """[1024, width] -> [4, 128, 2, width] DoubleRow layout."""
    return np.ascontiguousarray(
        a.reshape(4, 2, 128, width).transpose(0, 2, 1, 3))


def _make_in_maps(x, W_qkv, b_qkv, W_proj):
    masks_np = _host_masks()
    in_maps = []
    for core in range(N_CORES):
        b, g = core // 2, core % 2
        qc = slice(g * DL, (g + 1) * DL)
        kc = slice(D + g * DL, D + (g + 1) * DL)
        vc = slice(2 * D + g * DL, 2 * D + (g + 1) * DL)
        Wq = np.concatenate([W_qkv[:, qc], W_qkv[:, kc]], axis=1)  # [D,1024]
        Wv32 = W_qkv[:, vc] * np.float32(32.0)                     # [D, 512]
        Wp = W_proj[g * DL:(g + 1) * DL, :]                        # [512,1024]
        xT = np.ascontiguousarray(x[b].T)                          # [D, T]
        in_maps.append({
            "xbf": np.ascontiguousarray(xT[:, 0:TQ].astype(BFNP)),
            "xf8": _pair4(xT.astype(E4NP), T),
            "wqkbf": np.ascontiguousarray(Wq.astype(BFNP)),
            "wqk8": _pair4((Wq * np.float32(32.0)).astype(E4NP), 2 * DL),
            "wvbf": np.ascontiguousarray(Wv32.astype(BFNP)),
            "wv8": _pair4(Wv32.astype(E4NP), DL),
            "wpbf": np.ascontiguousarray(Wp.astype(BFNP)),
            "wp8": np.ascontiguousarray(
                (Wp * np.float32(32.0)).astype(E4NP)
                .reshape(2, 2, 128, D).transpose(0, 2, 1, 3)),
            "bqk": np.ascontiguousarray(
                np.concatenate([b_qkv[qc], b_qkv[kc]]).reshape(8, 128, 1)),
            "bv": np.ascontiguousarray(
                (b_qkv[vc] * np.float32(32.0)).reshape(1, DL)),
            "masks": masks_np,
        })
    return in_maps


def kernel(x, mask, W_qkv, b_qkv, W_proj, b_proj):
    x = np.asarray(x, dtype=np.float32)
    mask2d = np.asarray(mask, dtype=np.int32).reshape(T, T)
    W_qkv = np.asarray(W_qkv, dtype=np.float32)
    b_qkv = np.asarray(b_qkv, dtype=np.float32)
    W_proj = np.asarray(W_proj, dtype=np.float32)
    b_proj = np.asarray(b_proj, dtype=np.float32)

    if np.array_equal(mask2d, np.tril(np.ones((T, T), dtype=np.int32))):
        causal = True
    elif np.all(mask2d == 1):
        causal = False
    else:
        raise NotImplementedError("only causal (tril) or all-ones masks")

    nc = _get_nc(causal)
    in_maps = _make_in_maps(x, W_qkv, b_qkv, W_proj)
    res = run_bass_kernel_spmd(nc, in_maps, core_ids=list(range(N_CORES)))
    inv32 = np.float32(1.0 / 32.0)
    out = np.empty((B, T, D), dtype=np.float32)
    for b in range(B):
        acc = res.results[2 * b]["out"] + res.results[2 * b + 1]["out"]
        if causal:
            acc[TQ:] *= inv32
        else:
            acc *= inv32
        out[b] = acc + b_proj[None, :]
    return out


# revision 43
# speedup vs baseline: 1.0447x; 1.0062x over previous
"""Multi-head causal self-attention for TRN2, 8 NeuronCores.

Sharding: core i handles (batch b = i//2, head-group g = i%2); each head-group
is 8 of the 16 heads (local dims DL=512).  Computed in "transposed" space
(no on-device transposes).

Speed structure (v2):
  * fp8e4 DoubleRow matmuls (2 contraction rows/cycle) for the QKV
    projection, PV, and output projection of blocks j>=1; block j=0
    (queries+keys 0..511) stays bf16 because with few attended keys fp8
    quantization noise doesn't average out (rel-err budget).
  * Q^T/K^T stored head-pair packed ([128, T]: rows 0:64 = head 2i, rows
    64:128 = head 2i+1).  QK^T runs per-head with K=64 contraction; the
    two heads of a pair alternate PE row-strips so consecutive matmuls
    execute concurrently (row tiling) -> ~2x QK throughput.
  * V staged pre-scaled by 32 with a 32-valued tail column so the softmax
    row-sum rides the PV matmul ([V_e|32|V_o|32] fp8 chunk-pairs for
    DoubleRow).  exp uses bias -ln16 to keep e^s in fp8e4 range; all
    scales cancel in the normalize.  W_proj is staged *32 for the fp8
    path; the host divides rows >=512 by 32 at the end.
  * QKV of block j+1 and the projection of block j-1 interleave into
    attention block j so the PE never idles behind the ACT-bound softmax.
  * All dtype casts happen on the host (bf16/fp8 DMA); V staging and Q/K
    PSUM copy-outs are single strided DVE ops; reciprocals batched 8 heads
    at a time; mask-muls split DVE/GpSimd; projection copy-outs on GpSimd.
"""

import numpy as np
import ml_dtypes
from contextlib import ExitStack

import concourse.bass as bass
import concourse.mybir as mybir
import concourse.tile as tile
from concourse import bacc
from concourse.bass_utils import run_bass_kernel_spmd

B, T, D, H = 4, 2048, 1024, 16
DK = 64            # head dim
HL = 8             # heads per core
DL = HL * DK       # 512 local head dims per core
N_CORES = 8

F32 = mybir.dt.float32
F32R = mybir.dt.float32r
BF16 = mybir.dt.bfloat16
F8 = mybir.dt.float8e4
EXP = mybir.ActivationFunctionType.Exp
DR = mybir.MatmulPerfMode.DoubleRow
MULT = mybir.AluOpType.mult
ADD = mybir.AluOpType.add

E4NP = ml_dtypes.float8_e4m3
BFNP = ml_dtypes.bfloat16

TQ = 512           # query block size
NQB = T // TQ      # 4
NP = T // 256      # 8 key chunk-pairs
LN16 = float(np.log(16.0))

_CACHE = {}


def _build(causal: bool):
    nc = bacc.Bacc("TRN2", target_bir_lowering=False, debug=False,
                   num_devices=N_CORES)
    xbf_d = nc.dram_tensor("xbf", [D, TQ], BF16, kind="ExternalInput").ap()
    xf8_d = nc.dram_tensor("xf8", [4, 128, 2, T], F8,
                           kind="ExternalInput").ap()
    wqkbf_d = nc.dram_tensor("wqkbf", [D, 2 * DL], BF16,
                             kind="ExternalInput").ap()
    wqk8_d = nc.dram_tensor("wqk8", [4, 128, 2, 2 * DL], F8,
                            kind="ExternalInput").ap()
    wvbf_d = nc.dram_tensor("wvbf", [D, DL], BF16, kind="ExternalInput").ap()
    wv8_d = nc.dram_tensor("wv8", [4, 128, 2, DL], F8,
                           kind="ExternalInput").ap()
    wpbf_d = nc.dram_tensor("wpbf", [DL, D], BF16, kind="ExternalInput").ap()
    wp8_d = nc.dram_tensor("wp8", [2, 128, 2, D], F8,
                           kind="ExternalInput").ap()
    bqk_d = nc.dram_tensor("bqk", [8, 128, 1], F32, kind="ExternalInput").ap()
    bv_d = nc.dram_tensor("bv", [1, DL], F32, kind="ExternalInput").ap()
    masks_d = nc.dram_tensor("masks", [128, 4 * TQ], BF16,
                             kind="ExternalInput").ap()
    out_d = nc.dram_tensor("out", [T, D], F32, kind="ExternalOutput").ap()

    with tile.TileContext(nc) as tc, ExitStack() as top:
        persist = top.enter_context(tc.tile_pool(name="persist", bufs=1))

        # head-pair packed Q^T/K^T: rows 0:64 head 2i, rows 64:128 head 2i+1
        tileQ = [persist.tile([128, T], BF16, tag=f"tq{i}", name=f"tq{i}")
                 for i in range(4)]
        tileK = [persist.tile([128, T], BF16, tag=f"tk{i}", name=f"tk{i}")
                 for i in range(4)]
        # fp8 V chunk-pairs: (p, par, h, m) = [V_h | 32 | pad], *32-scaled
        vs2 = [persist.tile([128, 2, HL, 80], F8, tag=f"v2_{c}",
                            name=f"v2_{c}") for c in range(NP)]
        # bf16 V for block 0 (causal only): (p, h, m) = [V_h | 32]
        vs_bf = [persist.tile([128, HL, 65], BF16, tag=f"vb{t}",
                              name=f"vb{t}") for t in range(4)]
        yT_bf = [persist.tile([128, TQ], BF16, tag=f"yb{g}", name=f"yb{g}")
                 for g in range(4)]
        yTd = [persist.tile([128, 2, T], F8, tag=f"yd{g}",
                            name=f"yd{g}") for g in range(2)]
        wqk_bf = [persist.tile([128, 2 * DL], BF16, tag=f"wqb{d}",
                               name=f"wqb{d}") for d in range(8)]
        wqk8 = [persist.tile([128, 2, 2 * DL], F8, tag=f"wq8{g}",
                             name=f"wq8{g}") for g in range(4)]
        wv_bf = [persist.tile([128, DL], BF16, tag=f"wvb{d}",
                              name=f"wvb{d}") for d in range(8)]
        wv8 = [persist.tile([128, 2, DL], F8, tag=f"wv8{g}",
                            name=f"wv8{g}") for g in range(4)]
        wp_bf = [persist.tile([128, D], BF16, tag=f"wpb{g}",
                              name=f"wpb{g}") for g in range(4)]
        wp8 = [persist.tile([128, 2, D], F8, tag=f"wp8{g}",
                            name=f"wp8{g}") for g in range(2)]
        maskp = persist.tile([128, 4, TQ], BF16, tag="maskp", name="maskp")
        bqk_sb = [persist.tile([128, 1], F32, tag=f"bqk{m}", name=f"bqk{m}")
                  for m in range(8)]
        bv_r = persist.tile([1, DL], F32R, tag="bv_r", name="bv_r")
        ones_r = persist.tile([1, 128], F32R, tag="ones_r", name="ones_r")
        nln16 = persist.tile([128, 1], F32, tag="nln16", name="nln16")

        initp = top.enter_context(tc.tile_pool(name="initp", bufs=1))
        xstage = top.enter_context(tc.tile_pool(name="xstage", bufs=1))
        ppool = top.enter_context(tc.tile_pool(name="ppool", bufs=1))
        npool = top.enter_context(tc.tile_pool(name="npool", bufs=2))
        opool = top.enter_context(tc.tile_pool(name="opool", bufs=3))
        ps_a = top.enter_context(tc.tile_pool(name="ps_a", bufs=2,
                                              space="PSUM"))
        ps_s = top.enter_context(tc.tile_pool(name="ps_s", bufs=2,
                                              space="PSUM"))
        ps_o = top.enter_context(tc.tile_pool(name="ps_o", bufs=1,
                                              space="PSUM"))

        # ---------------- one-time init (j0-critical DMAs first) -----------
        for d in range(8):
            eng = nc.gpsimd if d % 2 == 0 else nc.sync
            eng.dma_start(wqk_bf[d][:],
                          wqkbf_d[d * 128:(d + 1) * 128, :])
        for d in range(8):
            nc.scalar.dma_start(wv_bf[d][:], wvbf_d[d * 128:(d + 1) * 128, :])
        for m in range(8):
            nc.gpsimd.dma_start(bqk_sb[m][:], bqk_d[m:m + 1, :, :])
        bv_f = initp.tile([1, DL], F32, tag="bv_f", name="bv_f")
        nc.gpsimd.dma_start(bv_f[:], bv_d)
        if causal:
            nc.gpsimd.dma_start(maskp[:, 0:4, :], masks_d)
        for g in range(4):
            nc.scalar.dma_start(wqk8[g][:], wqk8_d[g:g + 1, :, :, :])
            nc.scalar.dma_start(wv8[g][:], wv8_d[g:g + 1, :, :, :])
        for g in range(4):
            nc.gpsimd.dma_start(wp_bf[g][:], wpbf_d[g * 128:(g + 1) * 128, :])
        for g in range(2):
            nc.scalar.dma_start(wp8[g][:], wp8_d[g:g + 1, :, :, :])
        nc.vector.memset(nln16[:], -LN16)
        ones_f = initp.tile([1, 128], F32, tag="ones_f", name="ones_f")
        nc.vector.memset(ones_f[:], 1.0)
        nc.vector.tensor_copy(ones_r[:], ones_f[:])
        nc.vector.tensor_copy(bv_r[:], bv_f[:])
        for c in range(NP):
            nc.vector.memset(vs2[c][:, :, :, 64:65], 32.0)
        for t in range(4):
            nc.vector.memset(vs_bf[t][:, :, 64:65], 32.0)

        # ---------------- phase-1 step emitters ----------------
        def ph1_steps(j):
            jsl = slice(j * TQ, (j + 1) * TQ)
            steps = []
            bf = causal and j == 0
            if bf:
                xr = [xstage.tile([128, TQ], BF16, tag=f"xb{d}",
                                  name=f"xb{d}") for d in range(8)]

                def dma_x():
                    for d in range(8):
                        nc.sync.dma_start(xr[d][:],
                                          xbf_d[d * 128:(d + 1) * 128, :])
            else:
                xr = [xstage.tile([128, 2, TQ], F8, tag=f"x8{g}", bufs=2,
                                  name=f"x8{g}_{j}") for g in range(4)]

                def dma_x():
                    for g in range(4):
                        nc.sync.dma_start(xr[g][:], xf8_d[g:g + 1, :, :, jsl])
            steps.append(dma_x)

            def qk_tile(m):
                def emit():
                    ps = ps_a.tile([128, TQ], F32, tag="ps",
                                   name=f"psqk{j}_{m}")
                    if bf:
                        for d in range(8):
                            nc.tensor.matmul(
                                ps[:], wqk_bf[d][:, m * 128:(m + 1) * 128],
                                xr[d][:], start=(d == 0), stop=(d == 7))
                    else:
                        for g in range(4):
                            nc.tensor.matmul(
                                ps[:], wqk8[g][:, :, m * 128:(m + 1) * 128],
                                xr[g][:], start=(g == 0), stop=(g == 3),
                                perf_mode=DR)
                    dst = tileQ[m] if m < 4 else tileK[m - 4]
                    if bf:
                        nc.vector.tensor_scalar(
                            dst[:, jsl], ps[:], bqk_sb[m][:], None, op0=ADD)
                    else:
                        nc.vector.tensor_scalar(
                            dst[:, jsl], ps[:], 1.0 / 32.0, bqk_sb[m][:],
                            op0=MULT, op1=ADD)
                return emit

            for m in range(8):
                steps.append(qk_tile(m))

            def v_tile(tt):
                def emit():
                    c = tt % 4
                    ps = ps_a.tile([128, HL, 64], F32, tag="ps",
                                   name=f"psv{tt}")
                    if bf:
                        for d in range(8):
                            nc.tensor.matmul(
                                ps[:], xr[d][:, c * 128:(c + 1) * 128],
                                wv_bf[d][:], start=(d == 0), stop=False)
                    else:
                        for g in range(4):
                            nc.tensor.matmul(
                                ps[:], xr[g][:, :, c * 128:(c + 1) * 128],
                                wv8[g][:], start=(g == 0), stop=False,
                                perf_mode=DR)
                    nc.tensor.matmul(ps[:], ones_r[:, 0:128], bv_r[:],
                                     start=False, stop=True)
                    par = tt % 2
                    nc.vector.tensor_copy(
                        vs2[tt // 2][:, par:par + 1, :, 0:64], ps[:])
                    if causal and tt < 4:
                        nc.vector.tensor_copy(vs_bf[tt][:, :, 0:64], ps[:])
                return emit

            for tt in range(4 * j, 4 * j + 4):
                steps.append(v_tile(tt))
            return steps

        # ---------------- output projection steps ----------------
        def proj_step(t, nb):
            def emit():
                nsl = slice(nb * 512, (nb + 1) * 512)
                ps = ps_a.tile([128, TQ], F32, tag="ps", name=f"ps3_{t}_{nb}")
                if causal and t < 4:
                    for g in range(4):
                        nc.tensor.matmul(
                            ps[:], yT_bf[g][:, t * 128:(t + 1) * 128],
                            wp_bf[g][:, nsl], start=(g == 0), stop=(g == 3))
                else:
                    toff = t * 128
                    for g in range(2):
                        nc.tensor.matmul(
                            ps[:], yTd[g][:, :, toff:toff + 128],
                            wp8[g][:, :, nsl], start=(g == 0), stop=(g == 1),
                            perf_mode=DR)
                ot = opool.tile([128, TQ], F32, tag="ot", name=f"ot{t}_{nb}")
                nc.vector.tensor_copy(ot[:], ps[:])
                nc.sync.dma_start(out_d[t * 128:(t + 1) * 128, nsl], ot[:])
            return emit

        # ---------------- main fused loop ----------------
        for fn in ph1_steps(0):
            fn()
        pending = []
        for j in range(NQB):
            jsl = slice(j * TQ, (j + 1) * TQ)
            bf = causal and j == 0
            npairs = 2 * (j + 1) if causal else NP
            newsteps = ph1_steps(j + 1) if j + 1 < NQB else []
            pj = {0: [], 1: [], 2: [0], 3: [1, 2]}[j]
            projs = [proj_step(t, nb) for jj in pj
                     for t in range(4 * jj, 4 * jj + 4) for nb in range(2)]
            mixed = []
            while newsteps or projs:
                if newsteps:
                    mixed.append(newsteps.pop(0))
                if projs:
                    mixed.append(projs.pop(0))
            pending += mixed
            niter = 4 * npairs
            rate = len(pending) / max(1, niter)
            acc = 0.0

            for i in range(4):
                hA, hB = 2 * i, 2 * i + 1
                poA = ps_o.tile([128, TQ], F32, tag="poA", name=f"poA{j}_{i}")
                poB = ps_o.tile([128, TQ], F32, tag="poB", name=f"poB{j}_{i}")
                pends = {hA: None, hB: None}
                for ci in range(npairs):
                    ke = slice(ci * 256, ci * 256 + 128)
                    ko = slice(ci * 256 + 128, ci * 256 + 256)
                    diag = causal and ci >= 2 * j
                    r = ci - 2 * j
                    # fully-masked query-column prefixes on diagonal chunks
                    sk_e = 128 * 2 * r if diag else 0
                    sk_o = 128 * (2 * r + 1) if diag else 0
                    je = slice(j * TQ + sk_e, (j + 1) * TQ)
                    jo = slice(j * TQ + sk_o, (j + 1) * TQ)
                    ssA = ps_s.tile([128, 2, TQ], F32, tag="ss",
                                    name=f"ssA{j}_{i}_{ci}")
                    ssB = ps_s.tile([128, 2, TQ], F32, tag="ss",
                                    name=f"ssB{j}_{i}_{ci}")
                    nc.tensor.matmul(ssA[:, 0:1, sk_e:], tileK[i][0:64, ke],
                                     tileQ[i][0:64, je],
                                     start=True, stop=True)
                    nc.tensor.matmul(ssB[:, 0:1, sk_e:], tileK[i][64:128, ke],
                                     tileQ[i][64:128, je],
                                     start=True, stop=True)
                    nc.tensor.matmul(ssA[:, 1:2, sk_o:], tileK[i][0:64, ko],
                                     tileQ[i][0:64, jo],
                                     start=True, stop=True)
                    nc.tensor.matmul(ssB[:, 1:2, sk_o:], tileK[i][64:128, ko],
                                     tileQ[i][64:128, jo],
                                     start=True, stop=True)
                    kind = (BF16 if bf else F8)
                    ptA = ppool.tile([128, 2, TQ], kind,
                                     tag="ptb" if bf else "pt8",
                                     bufs=4 if bf else 8,
                                     name=f"ptA{j}_{i}_{ci}")
                    ptB = ppool.tile([128, 2, TQ], kind,
                                     tag="ptb" if bf else "pt8",
                                     bufs=4 if bf else 8,
                                     name=f"ptB{j}_{i}_{ci}")
                    acc += rate / 2
                    while acc >= 1.0 and pending:
                        pending.pop(0)()
                        acc -= 1.0
                    po_ = 256 * r if diag else 0
                    nc.scalar.activation(ptA[:, 0:2, po_:],
                                         ssA[:, 0:2, po_:],
                                         EXP, bias=nln16[:], scale=0.125)
                    nc.scalar.activation(ptB[:, 0:2, po_:],
                                         ssB[:, 0:2, po_:],
                                         EXP, bias=nln16[:], scale=0.125)
                    if diag:
                        ce_, co_ = 128 * (2 * r + 1), 128 * (2 * r + 2)
                        for pt in (ptA, ptB):
                            nc.vector.tensor_mul(pt[:, 0:1, po_:ce_],
                                                 pt[:, 0:1, po_:ce_],
                                                 maskp[:, 2 * r:2 * r + 1,
                                                       po_:ce_])
                            nc.vector.tensor_mul(pt[:, 1:2, po_:co_],
                                                 pt[:, 1:2, po_:co_],
                                                 maskp[:, 2 * r + 1:
                                                       2 * r + 2, po_:co_])
                    st = (ci == 1)
                    for po, h in ((poA, hA), (poB, hB)):
                        pend = pends[h]
                        if pend is None:
                            continue
                        pc, ppt = pend
                        if bf:
                            for e in range(2):
                                sk = 128 * (2 * pc + e) if causal else 0
                                nc.tensor.matmul(
                                    po[0:65, sk:],
                                    vs_bf[2 * pc + e][:, h:h + 1, :],
                                    ppt[:, e:e + 1, sk:],
                                    start=(st and e == 0), stop=False)
                        else:
                            pk = 256 if (causal and pc == 2 * j + 1) else 0
                            nc.tensor.matmul(
                                po[0:65, pk:],
                                vs2[pc][:, :, h:h + 1, 0:65],
                                ppt[:, 0:2, pk:], start=st, stop=False,
                                perf_mode=DR)
                    acc += rate / 2
                    while acc >= 1.0 and pending:
                        pending.pop(0)()
                        acc -= 1.0
                    pends[hA] = (ci, ptA)
                    pends[hB] = (ci, ptB)
                one = (npairs == 1)
                for po, h in ((poA, hA), (poB, hB)):
                    pc, ppt = pends[h]
                    if bf:
                        for e in range(2):
                            sk = 128 * (2 * pc + e) if causal else 0
                            nc.tensor.matmul(
                                po[0:65, sk:],
                                vs_bf[2 * pc + e][:, h:h + 1, :],
                                ppt[:, e:e + 1, sk:],
                                start=(one and e == 0), stop=(e == 1))
                    else:
                        pk = 256 if (causal and pc == 2 * j + 1) else 0
                        nc.tensor.matmul(
                            po[0:65, pk:],
                            vs2[pc][:, :, h:h + 1, 0:65],
                            ppt[:, 0:2, pk:], start=one, stop=True,
                            perf_mode=DR)
                    sums = npool.tile([1, TQ], F32, tag="sums",
                                      name=f"sm{j}_{h}", bufs=3)
                    nc.vector.tensor_copy(sums[:], po[64:65, :])
                    o_sb = npool.tile([64, TQ], BF16, tag="osb",
                                      name=f"ob{j}_{h}", bufs=3)
                    nc.vector.tensor_copy(o_sb[:], po[0:64, :])
                    recip = npool.tile([1, TQ], F32, tag="recip",
                                       name=f"rc{j}_{h}", bufs=3)
                    nc.vector.reciprocal_approx_fast(out=recip[:],
                                                     in_=sums[:])
                    recip_r = npool.tile([1, TQ], F32R, tag="recipr",
                                         name=f"rr{j}_{h}", bufs=3)
                    nc.vector.tensor_copy(recip_r[:], recip[:])
                    pb = ps_a.tile([64, TQ], F32, tag="ps",
                                   name=f"pb{j}_{h}")
                    nc.tensor.matmul(pb[:], ones_r[:, 0:64], recip_r[:],
                                     start=True, stop=True)
                    if bf:
                        nc.vector.tensor_mul(
                            yT_bf[h // 2][(h % 2) * 64:(h % 2) * 64 + 64, :],
                            o_sb[:], pb[:])
                    else:
                        gg, e = h // 4, (h // 2) % 2
                        nc.vector.tensor_mul(
                            yTd[gg][(h % 2) * 64:(h % 2) * 64 + 64,
                                    e:e + 1, jsl],
                            o_sb[:], pb[:])

            while pending:
                pending.pop(0)()
        for t in range(4 * (NQB - 1), 4 * NQB):
            for nb in range(2):
                proj_step(t, nb)()

    nc.compile()
    return nc


def _get_nc(causal: bool):
    if causal not in _CACHE:
        _CACHE[causal] = _build(causal)
    return _CACHE[causal]


def _host_masks() -> np.ndarray:
    i = np.arange(128)[:, None]
    jj = np.arange(TQ)[None, :]
    blocks = [(jj >= i + s * 128).astype(np.float32) for s in range(4)]
    return np.ascontiguousarray(
        np.concatenate(blocks, axis=1).astype(BFNP))


def _pair4(a, width):
    """[1024, width] -> [4, 128, 2, width] DoubleRow layout."""
    return np.ascontiguousarray(
        a.reshape(4, 2, 128, width).transpose(0, 2, 1, 3))


def _make_in_maps(x, W_qkv, b_qkv, W_proj):
    masks_np = _host_masks()
    in_maps = []
    for core in range(N_CORES):
        b, g = core // 2, core % 2
        qc = slice(g * DL, (g + 1) * DL)
        kc = slice(D + g * DL, D + (g + 1) * DL)
        vc = slice(2 * D + g * DL, 2 * D + (g + 1) * DL)
        Wq = np.concatenate([W_qkv[:, qc], W_qkv[:, kc]], axis=1)  # [D,1024]
        Wv32 = W_qkv[:, vc] * np.float32(32.0)                     # [D, 512]
        Wp = W_proj[g * DL:(g + 1) * DL, :]                        # [512,1024]
        xT = np.ascontiguousarray(x[b].T)                          # [D, T]
        in_maps.append({
            "xbf": np.ascontiguousarray(xT[:, 0:TQ].astype(BFNP)),
            "xf8": _pair4(xT.astype(E4NP), T),
            "wqkbf": np.ascontiguousarray(Wq.astype(BFNP)),
            "wqk8": _pair4((Wq * np.float32(32.0)).astype(E4NP), 2 * DL),
            "wvbf": np.ascontiguousarray(Wv32.astype(BFNP)),
            "wv8": _pair4(Wv32.astype(E4NP), DL),
            "wpbf": np.ascontiguousarray(Wp.astype(BFNP)),
            "wp8": np.ascontiguousarray(
                (Wp * np.float32(32.0)).astype(E4NP)
                .reshape(2, 2, 128, D).transpose(0, 2, 1, 3)),
            "bqk": np.ascontiguousarray(
                np.concatenate([b_qkv[qc], b_qkv[kc]]).reshape(8, 128, 1)),
            "bv": np.ascontiguousarray(
                (b_qkv[vc] * np.float32(32.0)).reshape(1, DL)),
            "masks": masks_np,
        })
    return in_maps


def kernel(x, mask, W_qkv, b_qkv, W_proj, b_proj):
    x = np.asarray(x, dtype=np.float32)
    mask2d = np.asarray(mask, dtype=np.int32).reshape(T, T)
    W_qkv = np.asarray(W_qkv, dtype=np.float32)
    b_qkv = np.asarray(b_qkv, dtype=np.float32)
    W_proj = np.asarray(W_proj, dtype=np.float32)
    b_proj = np.asarray(b_proj, dtype=np.float32)

    if np.array_equal(mask2d, np.tril(np.ones((T, T), dtype=np.int32))):
        causal = True
    elif np.all(mask2d == 1):
        causal = False
    else:
        raise NotImplementedError("only causal (tril) or all-ones masks")

    nc = _get_nc(causal)
    in_maps = _make_in_maps(x, W_qkv, b_qkv, W_proj)
    res = run_bass_kernel_spmd(nc, in_maps, core_ids=list(range(N_CORES)))
    inv32 = np.float32(1.0 / 32.0)
    out = np.empty((B, T, D), dtype=np.float32)
    for b in range(B):
        acc = res.results[2 * b]["out"] + res.results[2 * b + 1]["out"]
        if causal:
            acc[TQ:] *= inv32
        else:
            acc *= inv32
        out[b] = acc + b_proj[None, :]
    return out
